# revision 5
# baseline (speedup 1.0000x reference)
"""Trainium2 Bass kernel for multi-head self-attention with RoPE (causal).

Problem: B=2, S=2048, D=1024, H=16 heads, dk=64, fp32.

Sharding (8 NeuronCores): core c handles batch b = c//4 and head group
g = c%4 (heads 4g..4g+3).  Data parallel over batch, tensor parallel over
heads: w_q/w_k/w_v rows and w_o columns are split by head group; each core
produces a partial output [S, D] (its heads' contribution through w_o) and
the host sums the 4 partials per batch.

Device algorithm per core (all matmuls in float32r — fp32 bits, reduced
precision multiplier, 4x the fp32 matmul rate):
  1. q|k|v projections token-major: out[128 tok, 512/256] tiles, contracting
     over D in 8 chunks of 128.  q/k weight rows are pre-permuted per head
     (even pairs first) so RoPE uses contiguous halves.
  2. RoPE applied in token-major layout on DVE straight out of PSUM, then
     PE-transposed to feature-major qT/kT [64 feats, S] per head.
  3. Attention per head with scores TRANSPOSED: scoresT[k, q] tiles
     (k on partitions), exp on ScalarE (scale=1/8 folded in, no max pass —
     inputs are scale-bounded), causal via block skipping + one triangular
     mask multiply on diagonal blocks, then attnT[dk, q] accumulated in
     PSUM with contraction over k.  A ones-row appended to V gives the
     softmax denominator as row 64 of the same accumulation.
  4. Normalize: reciprocal of the denominator row, PE outer-product
     broadcast, fused into the PSUM->SBUF copy of attnT.
  5. Output projection from attnT (feature-major is exactly the lhsT the
     PE wants): out[128 tok, 512] accumulating the 4 heads.
"""

import sys

sys.path.insert(0, "/opt/trn_rl_repo")

import numpy as np

B, S, D, H, DK = 2, 2048, 1024, 16, 64
NCORES = 8
HPC = 4            # heads per core
F = HPC * DK       # 256 features per core
ID = 128           # partition tile
NT = S // ID       # 16 token tiles
KC = D // ID       # 8 contraction chunks
QB = 512           # q block width (one PSUM bank of fp32)
NQB = S // QB      # 4

_prog = None


def _build_program():
    import concourse.bass as bass
    import concourse.tile as tile
    from concourse import bacc, mybir
    from concourse.masks import make_identity

    F32 = mybir.dt.float32
    F32R = mybir.dt.float32r
    AF = mybir.ActivationFunctionType
    MUL = mybir.AluOpType.mult

    nc = bacc.Bacc("TRN2", target_bir_lowering=False, debug=False)

    xT = nc.declare_dram_parameter("xT", [D, S], F32R, isOutput=False)
    wqkv = nc.declare_dram_parameter("wqkv", [D, 3 * F], F32R, isOutput=False)
    wo01 = nc.declare_dram_parameter("wo01", [ID, D], F32R, isOutput=False)
    wo23 = nc.declare_dram_parameter("wo23", [ID, D], F32R, isOutput=False)
    cosr = nc.declare_dram_parameter("cosr", [S, ID], F32, isOutput=False)
    sinr = nc.declare_dram_parameter("sinr", [S, ID], F32, isOutput=False)
    trim = nc.declare_dram_parameter("trimask", [ID, ID], F32R, isOutput=False)
    out = nc.declare_dram_parameter("out", [S, D], F32, isOutput=True)

    with nc.allow_low_precision(reason="f32r matmul pipeline by design"), \
         tile.TileContext(nc) as tc:
        # ---------------- persistent pools ----------------
        with tc.tile_pool(name="const", bufs=1) as constp, \
             tc.tile_pool(name="wo", bufs=1) as wop, \
             tc.tile_pool(name="qkT", bufs=1) as qkTp, \
             tc.tile_pool(name="vaug", bufs=1) as vaugp, \
             tc.tile_pool(name="attnT", bufs=1) as attnTp:

            ident = constp.tile([ID, ID], F32, tag="ident")
            make_identity(nc, ident[:])
            trimask = constp.tile([ID, ID], F32R, tag="trimask")
            nc.sync.dma_start(trimask[:], trim[:])
            ones_f = constp.tile([1, DK], F32, tag="ones_f")
            nc.vector.memset(ones_f[:], 1.0)
            ones_r = constp.tile([1, DK], F32R, tag="ones_r")
            nc.vector.tensor_copy(ones_r[:], ones_f[:])
            onecol_f = constp.tile([ID, NT], F32, tag="onecol")
            nc.vector.memset(onecol_f[:], 1.0)

            wo_sb = [wop.tile([DK, D], F32R, tag=f"wo{h}", name=f"wo_sb{h}") for h in range(HPC)]
            for h in range(HPC):
                src_p = wo01 if h < 2 else wo23
                nc.sync.dma_start(
                    wo_sb[h][:], src_p[DK * (h % 2) : DK * (h % 2) + DK, :]
                )

            # feature-major q/k, one tensor per head pair; head h lives on
            # partitions 64*(h%2) .. of tensor h//2
            qT = [qkTp.tile([ID, S], F32R, tag=f"qT{p}", name=f"qT{p}") for p in range(2)]
            kT = [qkTp.tile([ID, S], F32R, tag=f"kT{p}", name=f"kT{p}") for p in range(2)]

            # token-major v with a ones column per head: head h in columns
            # [65h, 65h+64], ones at column 65h+64
            vaug = vaugp.tile([ID, NT, HPC * (DK + 1)], F32R, tag="vaug")
            for h in range(HPC):
                nc.vector.tensor_copy(
                    vaug[:, :, 65 * h + DK], onecol_f[:]
                )

            attnT_sb = [
                attnTp.tile([DK, S], F32R, tag=f"attnT{h}", name=f"attnT{h}")
                for h in range(HPC)
            ]

            # ---------------- phase 1+2: projections, rope, transposes ----
            with tc.tile_pool(name="wqkv", bufs=1) as wqkvp, \
                 tc.tile_pool(name="cs", bufs=1) as csp, \
                 tc.tile_pool(name="xtile", bufs=12) as xtp, \
                 tc.tile_pool(name="rot", bufs=3) as rotp, \
                 tc.tile_pool(name="tmp", bufs=4) as tmpp, \
                 tc.tile_pool(name="proj_ps", bufs=2, space="PSUM") as pjp, \
                 tc.tile_pool(name="vps", bufs=2, space="PSUM") as vpp, \
                 tc.tile_pool(name="tp_ps", bufs=3, space="PSUM") as tpp:

                wqkv_sb = wqkvp.tile([ID, KC, 3 * F], F32R, tag="wqkv")
                nc.sync.dma_start(
                    wqkv_sb[:],
                    wqkv[:].rearrange("(c p) f -> p c f", p=ID),
                )
                cos_sb = csp.tile([ID, NT, HPC, 32], F32, tag="cos")
                sin_sb = csp.tile([ID, NT, HPC, 32], F32, tag="sin")
                nc.sync.dma_start(
                    cos_sb[:].rearrange("p t h d -> p t (h d)"),
                    cosr[:].rearrange("(t p) f -> p t f", p=ID),
                )
                nc.sync.dma_start(
                    sin_sb[:].rearrange("p t h d -> p t (h d)"),
                    sinr[:].rearrange("(t p) f -> p t f", p=ID),
                )

                for mt in range(NT):
                    ts = slice(mt * ID, (mt + 1) * ID)
                    qk_ps = pjp.tile([ID, 2, HPC, DK], F32, tag="qk_ps")
                    v_ps = vpp.tile([ID, F], F32, tag="v_ps")
                    for kc in range(KC):
                        xt = xtp.tile([ID, ID], F32R, tag="xt")
                        nc.sync.dma_start(
                            xt[:], xT[kc * ID : (kc + 1) * ID, ts]
                        )
                        nc.tensor.matmul(
                            qk_ps[:].rearrange("p a h d -> p (a h d)"),
                            xt[:],
                            wqkv_sb[:, kc, 0 : 2 * F],
                            start=(kc == 0),
                            stop=(kc == KC - 1),
                        )
                        nc.tensor.matmul(
                            v_ps[:],
                            xt[:],
                            wqkv_sb[:, kc, 2 * F : 3 * F],
                            start=(kc == 0),
                            stop=(kc == KC - 1),
                        )

                    ct = cos_sb[:, mt]
                    st = sin_sb[:, mt]
                    # RoPE on DVE: halves are contiguous (weights were
                    # pre-permuted even-pairs-first per head)
                    for a, dst_name in ((0, "q"), (1, "k")):
                        src = qk_ps[:, a]            # [ID, HPC, DK] psum
                        rot = rotp.tile([ID, HPC, DK], F32, tag=f"rot{a}")
                        t1 = tmpp.tile([ID, HPC, 32], F32, tag=f"t1{a}")
                        t2 = tmpp.tile([ID, HPC, 32], F32, tag=f"t2{a}")
                        top = src[:, :, 0:32]
                        bot = src[:, :, 32:64]
                        nc.vector.tensor_tensor(
                            out=rot[:, :, 0:32], in0=top, in1=ct, op=MUL
                        )
                        nc.vector.tensor_tensor(
                            out=t1[:], in0=bot, in1=st, op=MUL
                        )
                        nc.vector.tensor_sub(
                            rot[:, :, 0:32], rot[:, :, 0:32], t1[:]
                        )
                        nc.vector.tensor_tensor(
                            out=rot[:, :, 32:64], in0=top, in1=st, op=MUL
                        )
                        nc.vector.tensor_tensor(
                            out=t2[:], in0=bot, in1=ct, op=MUL
                        )
                        nc.vector.tensor_add(
                            rot[:, :, 32:64], rot[:, :, 32:64], t2[:]
                        )
                        # transpose to feature-major, per head pair
                        for p in range(2):
                            tp = tpp.tile([ID, ID], F32, tag="tp")
                            nc.tensor.transpose(
                                tp[:],
                                rot[:, 2 * p : 2 * p + 2, :],
                                ident[:],
                            )
                            dst = (qT if a == 0 else kT)[p][:, ts]
                            if a == 0:
                                nc.scalar.copy(dst, tp[:])
                            else:
                                nc.vector.tensor_copy(dst, tp[:])

                    # v copies into augmented layout (ACT)
                    for h in range(HPC):
                        nc.scalar.copy(
                            vaug[:, mt, 65 * h : 65 * h + DK],
                            v_ps[:, DK * h : DK * h + DK],
                        )

            # ---------------- phase 3: attention per head ----------------
            with tc.tile_pool(name="expT", bufs=4) as expp, \
                 tc.tile_pool(name="den", bufs=2) as denp, \
                 tc.tile_pool(name="recb", bufs=2) as recbp, \
                 tc.tile_pool(name="sc_ps", bufs=3, space="PSUM") as scp, \
                 tc.tile_pool(name="attn_ps", bufs=1, space="PSUM") as atp:

                for h in range(HPC):
                    p, w = h // 2, h % 2
                    po = DK * w
                    attn_ps = atp.tile([DK + 1, S], F32, tag="attn_ps")
                    for kb in range(NT):
                        cd = kb // 4
                        for qb in range(cd, NQB):
                            qcs = qb * QB if qb > cd else kb * ID
                            wd = (qb + 1) * QB - qcs
                            sc = scp.tile([ID, QB], F32, tag="sc")
                            nc.tensor.matmul(
                                sc[:, 0:wd],
                                kT[p][po : po + DK, kb * ID : (kb + 1) * ID],
                                qT[p][po : po + DK, qcs : qcs + wd],
                                start=True,
                                stop=True,
                            )
                            ex = expp.tile([ID, QB], F32R, tag="ex")
                            nc.scalar.activation(
                                ex[:, 0:wd], sc[:, 0:wd], AF.Exp, scale=0.125
                            )
                            if qb == cd:
                                nc.vector.tensor_tensor(
                                    out=ex[:, 0:ID],
                                    in0=ex[:, 0:ID],
                                    in1=trimask[:],
                                    op=MUL,
                                )
                            nc.tensor.matmul(
                                attn_ps[:, qcs : qcs + wd],
                                vaug[:, kb, 65 * h : 65 * h + DK + 1],
                                ex[:, 0:wd],
                                start=(kb == 0),
                                stop=(kb == NT - 1),
                                skip_group_check=True,
                            )
                    # normalize rows 0..63 by row 64
                    den = denp.tile([1, S], F32, tag="den")
                    nc.scalar.copy(den[:], attn_ps[DK : DK + 1, :])
                    rec = denp.tile([1, S], F32R, tag="rec")
                    nc.vector.reciprocal(rec[:], den[:])
                    for qb in range(NQB):
                        qs = slice(qb * QB, (qb + 1) * QB)
                        bc = scp.tile([DK, QB], F32, tag="sc")
                        nc.tensor.matmul(
                            bc[:], ones_r[:], rec[:, qs], start=True, stop=True
                        )
                        rb = recbp.tile([DK, QB], F32, tag="rb")
                        nc.scalar.copy(rb[:], bc[:])
                        nc.vector.tensor_tensor(
                            out=attnT_sb[h][:, qs],
                            in0=attn_ps[0:DK, qs],
                            in1=rb[:],
                            op=MUL,
                        )

            # ---------------- phase 4: output projection ------------------
            with tc.tile_pool(name="osb", bufs=3) as osbp, \
                 tc.tile_pool(name="ops", bufs=2, space="PSUM") as opsp:
                for mt in range(NT):
                    ts = slice(mt * ID, (mt + 1) * ID)
                    for nb in range(2):
                        ns = slice(nb * QB, (nb + 1) * QB)
                        o_ps = opsp.tile([ID, QB], F32, tag="o_ps")
                        for h in range(HPC):
                            nc.tensor.matmul(
                                o_ps[:],
                                attnT_sb[h][:, ts],
                                wo_sb[h][:, ns],
                                start=(h == 0),
                                stop=(h == HPC - 1),
                            )
                        ob = osbp.tile([ID, QB], F32, tag="ob")
                        if nb == 0:
                            nc.scalar.copy(ob[:], o_ps[:])
                        else:
                            nc.vector.tensor_copy(ob[:], o_ps[:])
                        nc.sync.dma_start(out[ts, ns], ob[:])

    nc.compile()
    return nc


def _get_prog():
    global _prog
    if _prog is None:
        _prog = _build_program()
    return _prog


def _host_prep(x, w_q, w_k, w_v, w_o, sin_tab, cos_tab, token_positions):
    """Build the 8 per-core input maps."""
    x = np.asarray(x, dtype=np.float32)
    w_q = np.asarray(w_q, dtype=np.float32)
    w_k = np.asarray(w_k, dtype=np.float32)
    w_v = np.asarray(w_v, dtype=np.float32)
    w_o = np.asarray(w_o, dtype=np.float32)
    sin_tab = np.asarray(sin_tab, dtype=np.float32)
    cos_tab = np.asarray(cos_tab, dtype=np.float32)
    tpos = np.asarray(token_positions).astype(np.int64)

    # per-head row permutation: even rope lanes first, then odd
    perm = np.concatenate([np.arange(0, DK, 2), np.arange(1, DK, 2)])

    xT_b = [np.ascontiguousarray(x[b].T) for b in range(B)]
    cs_b = []
    for b in range(B):
        cosg = cos_tab[tpos[b]]             # [S, 32]
        sing = sin_tab[tpos[b]]
        cs_b.append(
            (
                np.ascontiguousarray(np.tile(cosg, (1, HPC))),
                np.ascontiguousarray(np.tile(sing, (1, HPC))),
            )
        )

    trimask = np.triu(np.ones((ID, ID), dtype=np.float32))

    in_maps = []
    for c in range(NCORES):
        b, g = divmod(c, HPC)
        rows = np.arange(g * F, (g + 1) * F)
        rows_qk = (rows.reshape(HPC, DK)[:, perm]).reshape(-1)
        wq_t = w_q[rows_qk].T               # [D, F]
        wk_t = w_k[rows_qk].T
        wv_t = w_v[rows].T
        wqkv_c = np.ascontiguousarray(
            np.concatenate([wq_t, wk_t, wv_t], axis=1)
        )
        wo_c = np.ascontiguousarray(w_o[:, rows].T)  # [F, D]
        in_maps.append(
            {
                "xT": xT_b[b],
                "wqkv": wqkv_c,
                "wo01": np.ascontiguousarray(wo_c[0:ID]),
                "wo23": np.ascontiguousarray(wo_c[ID : 2 * ID]),
                "cosr": cs_b[b][0],
                "sinr": cs_b[b][1],
                "trimask": trimask,
            }
        )
    return in_maps


def kernel(x, w_q, w_k, w_v, w_o, sin_tab, cos_tab, token_positions):
    from concourse.bass_utils import run_bass_kernel_spmd

    nc = _get_prog()
    in_maps = _host_prep(
        x, w_q, w_k, w_v, w_o, sin_tab, cos_tab, token_positions
    )
    res = run_bass_kernel_spmd(nc, in_maps, list(range(NCORES)))
    outs = [np.asarray(res.results[c]["out"]) for c in range(NCORES)]
    full = np.empty((B, S, D), dtype=np.float32)
    for b in range(B):
        full[b] = outs[b * HPC]
        for g in range(1, HPC):
            full[b] += outs[b * HPC + g]
    return full


if __name__ == "__main__":
    import reference

    inputs = {k: np.asarray(v) for k, v in reference.setup_inputs().items()}
    got = kernel(**inputs)
    exp = np.asarray(reference.reference(**reference.setup_inputs()))
    err = np.abs(got - exp).max() / np.abs(exp).max()
    print("Relative error:", err)


# revision 7
# speedup vs baseline: 1.2319x; 1.2319x over previous
"""Trainium2 Bass kernel for multi-head self-attention with RoPE (causal).

Problem: B=2, S=2048, D=1024, H=16 heads, dk=64, fp32.

Sharding (8 NeuronCores): core c handles batch b = c//4 and head group
g = c%4 (heads 4g..4g+3).  Data parallel over batch, tensor parallel over
heads: w_q/w_k/w_v rows and w_o columns are split by head group; each core
produces a partial output [S, D] (its heads' contribution through w_o) and
the host sums the 4 partials per batch.

Device algorithm per core (all matmuls in float32r — fp32 bits, reduced
precision multiplier, 4x the fp32 matmul rate):
  1. q|k|v projections token-major: out[128 tok, 512/256] tiles, contracting
     over D in 8 chunks of 128.  q/k weight rows are pre-permuted per head
     (even pairs first) so RoPE uses contiguous halves.
  2. RoPE applied in token-major layout on DVE straight out of PSUM, then
     PE-transposed to feature-major qT/kT [64 feats, S] per head.
  3. Attention per head with scores TRANSPOSED: scoresT[k, q] tiles
     (k on partitions), exp on ScalarE (scale=1/8 folded in, no max pass —
     inputs are scale-bounded), causal via block skipping + one triangular
     mask multiply on diagonal blocks, then attnT[dk, q] accumulated in
     PSUM with contraction over k.  A ones-row appended to V gives the
     softmax denominator as row 64 of the same accumulation.
  4. Normalize: PE outer-product broadcast of the denominator row,
     reciprocal_approx_fast on the broadcast, multiplied into the
     PSUM->SBUF copy of attnT.  Odd heads are shifted to partitions 64-127
     by an SBUF->SBUF DMA so the output projection contracts K=128.
  5. Output projection from packed attnT pairs (feature-major is exactly
     the lhsT the PE wants): out[128 tok, 512] accumulating 2 head pairs.
"""

import sys

sys.path.insert(0, "/opt/trn_rl_repo")

import numpy as np

B, S, D, H, DK = 2, 2048, 1024, 16, 64
NCORES = 8
HPC = 4            # heads per core
F = HPC * DK       # 256 features per core
ID = 128           # partition tile
NT = S // ID       # 16 token tiles
KC = D // ID       # 8 contraction chunks
QB = 512           # PSUM bank width in fp32
NQB = S // QB      # 4
XW = 512           # x DMA tile width (tokens)

_prog = None


def _build_program():
    import concourse.bass as bass
    import concourse.tile as tile
    from concourse import bacc, mybir
    from concourse.masks import make_identity

    F32 = mybir.dt.float32
    F32R = mybir.dt.float32r
    AF = mybir.ActivationFunctionType
    MUL = mybir.AluOpType.mult

    nc = bacc.Bacc("TRN2", target_bir_lowering=False, debug=False)

    xT = nc.declare_dram_parameter("xT", [D, S], F32R, isOutput=False)
    wqkv = nc.declare_dram_parameter("wqkv", [D, 3 * F], F32R, isOutput=False)
    wo01 = nc.declare_dram_parameter("wo01", [ID, D], F32R, isOutput=False)
    wo23 = nc.declare_dram_parameter("wo23", [ID, D], F32R, isOutput=False)
    cosr = nc.declare_dram_parameter("cosr", [S, ID], F32, isOutput=False)
    sinr = nc.declare_dram_parameter("sinr", [S, ID], F32, isOutput=False)
    trim = nc.declare_dram_parameter("trimask", [ID, ID], F32R, isOutput=False)
    out = nc.declare_dram_parameter("out", [S, D], F32, isOutput=True)

    with nc.allow_low_precision(reason="f32r matmul pipeline by design"), \
         tile.TileContext(nc) as tc:
        # ---------------- persistent pools ----------------
        with tc.tile_pool(name="const", bufs=1) as constp, \
             tc.tile_pool(name="wo", bufs=1) as wop, \
             tc.tile_pool(name="qkT", bufs=1) as qkTp, \
             tc.tile_pool(name="vaug", bufs=1) as vaugp, \
             tc.tile_pool(name="attnT", bufs=1) as attnTp:

            ident = constp.tile([ID, ID], F32, tag="ident")
            make_identity(nc, ident[:])
            trimask = constp.tile([ID, ID], F32R, tag="trimask")
            nc.sync.dma_start(trimask[:], trim[:])
            ones_f = constp.tile([1, DK], F32, tag="ones_f")
            nc.vector.memset(ones_f[:], 1.0)
            ones_r = constp.tile([1, DK], F32R, tag="ones_r")
            nc.vector.tensor_copy(ones_r[:], ones_f[:])
            onecol_f = constp.tile([ID, NT], F32, tag="onecol")
            nc.vector.memset(onecol_f[:], 1.0)

            wo_sb = [
                wop.tile([ID, D], F32R, tag=f"wo{p}", name=f"wo_sb{p}")
                for p in range(2)
            ]
            nc.sync.dma_start(wo_sb[0][:], wo01[:])
            nc.sync.dma_start(wo_sb[1][:], wo23[:])

            # feature-major q/k, one tensor per head pair; head h lives on
            # partitions 64*(h%2) .. of tensor h//2
            qT = [
                qkTp.tile([ID, S], F32R, tag=f"qT{p}", name=f"qT{p}")
                for p in range(2)
            ]
            kT = [
                qkTp.tile([ID, S], F32R, tag=f"kT{p}", name=f"kT{p}")
                for p in range(2)
            ]

            # token-major v with a ones column per head: head h in columns
            # [65h, 65h+64], ones at column 65h+64
            vaug = vaugp.tile([ID, NT, HPC * (DK + 1)], F32R, tag="vaug")
            for h in range(HPC):
                nc.vector.tensor_copy(vaug[:, :, 65 * h + DK], onecol_f[:])

            # packed attnT: pair p holds head 2p on partitions 0-63 and
            # head 2p+1 on partitions 64-127
            attnT_sb = [
                attnTp.tile([ID, S], F32R, tag=f"attnT{p}", name=f"attnT{p}")
                for p in range(2)
            ]
            # odd heads normalize into a low-partition temp, then an
            # SBUF->SBUF DMA shifts them up to partitions 64-127
            odd_tmp = attnTp.tile([DK, S], F32R, tag="odd_tmp")

            # ---------------- phase 1+2: projections, rope, transposes ----
            with tc.tile_pool(name="wqkv", bufs=1) as wqkvp, \
                 tc.tile_pool(name="cs", bufs=1) as csp, \
                 tc.tile_pool(name="xtile", bufs=16) as xtp, \
                 tc.tile_pool(name="rot", bufs=3) as rotp, \
                 tc.tile_pool(name="tmp", bufs=4) as tmpp, \
                 tc.tile_pool(name="proj_ps", bufs=2, space="PSUM") as pjp, \
                 tc.tile_pool(name="vps", bufs=2, space="PSUM") as vpp, \
                 tc.tile_pool(name="tp_ps", bufs=3, space="PSUM") as tpp:

                wqkv_sb = wqkvp.tile([ID, KC, 3 * F], F32R, tag="wqkv")
                nc.sync.dma_start(
                    wqkv_sb[:],
                    wqkv[:].rearrange("(c p) f -> p c f", p=ID),
                )
                cos_sb = csp.tile([ID, NT, HPC, 32], F32, tag="cos")
                sin_sb = csp.tile([ID, NT, HPC, 32], F32, tag="sin")
                nc.sync.dma_start(
                    cos_sb[:].rearrange("p t h d -> p t (h d)"),
                    cosr[:].rearrange("(t p) f -> p t f", p=ID),
                )
                nc.sync.dma_start(
                    sin_sb[:].rearrange("p t h d -> p t (h d)"),
                    sinr[:].rearrange("(t p) f -> p t f", p=ID),
                )

                xts = {}
                for mt in range(NT):
                    ts = slice(mt * ID, (mt + 1) * ID)
                    qk_ps = pjp.tile([ID, 2, HPC, DK], F32, tag="qk_ps")
                    v_ps = vpp.tile([ID, F], F32, tag="v_ps")
                    sub = mt % (XW // ID)
                    for kc in range(KC):
                        if sub == 0:
                            xt = xtp.tile([ID, XW], F32R, tag="xt",
                                          name=f"xt{mt}_{kc}")
                            nc.gpsimd.dma_start(
                                xt[:],
                                xT[kc * ID : (kc + 1) * ID,
                                   mt * ID : mt * ID + XW],
                            )
                            xts[kc] = xt
                        xsl = xts[kc][:, sub * ID : (sub + 1) * ID]
                        nc.tensor.matmul(
                            qk_ps[:].rearrange("p a h d -> p (a h d)"),
                            xsl,
                            wqkv_sb[:, kc, 0 : 2 * F],
                            start=(kc == 0),
                            stop=(kc == KC - 1),
                        )
                        nc.tensor.matmul(
                            v_ps[:],
                            xsl,
                            wqkv_sb[:, kc, 2 * F : 3 * F],
                            start=(kc == 0),
                            stop=(kc == KC - 1),
                        )

                    ct = cos_sb[:, mt]
                    st = sin_sb[:, mt]
                    # RoPE on DVE: halves are contiguous (weights were
                    # pre-permuted even-pairs-first per head)
                    for a in range(2):          # 0 = q, 1 = k
                        src = qk_ps[:, a]       # [ID, HPC, DK] psum
                        rot = rotp.tile([ID, HPC, DK], F32, tag=f"rot{a}")
                        t1 = tmpp.tile([ID, HPC, 32], F32, tag=f"t1{a}")
                        t2 = tmpp.tile([ID, HPC, 32], F32, tag=f"t2{a}")
                        top = src[:, :, 0:32]
                        bot = src[:, :, 32:64]
                        nc.vector.tensor_tensor(
                            out=rot[:, :, 0:32], in0=top, in1=ct, op=MUL
                        )
                        nc.vector.tensor_tensor(
                            out=t1[:], in0=bot, in1=st, op=MUL
                        )
                        nc.vector.tensor_sub(
                            rot[:, :, 0:32], rot[:, :, 0:32], t1[:]
                        )
                        nc.vector.tensor_tensor(
                            out=rot[:, :, 32:64], in0=top, in1=st, op=MUL
                        )
                        nc.vector.tensor_tensor(
                            out=t2[:], in0=bot, in1=ct, op=MUL
                        )
                        nc.vector.tensor_add(
                            rot[:, :, 32:64], rot[:, :, 32:64], t2[:]
                        )
                        # transpose to feature-major, per head pair
                        for p in range(2):
                            tp = tpp.tile([ID, ID], F32, tag="tp")
                            nc.tensor.transpose(
                                tp[:],
                                rot[:, 2 * p : 2 * p + 2, :],
                                ident[:],
                            )
                            dst = (qT if a == 0 else kT)[p][:, ts]
                            if a == 0:
                                nc.scalar.copy(dst, tp[:])
                            else:
                                nc.vector.tensor_copy(dst, tp[:])

                    # v copies into augmented layout (ACT)
                    for h in range(HPC):
                        nc.scalar.copy(
                            vaug[:, mt, 65 * h : 65 * h + DK],
                            v_ps[:, DK * h : DK * h + DK],
                        )

            # ---------------- phase 3: attention per head ----------------
            with tc.tile_pool(name="expT", bufs=3) as expp, \
                 tc.tile_pool(name="den", bufs=2) as denp, \
                 tc.tile_pool(name="recb", bufs=2) as recbp, \
                 tc.tile_pool(name="sc_ps", bufs=2, space="PSUM") as scp, \
                 tc.tile_pool(name="attn_ps", bufs=1, space="PSUM") as atp:

                for h in range(HPC):
                    p, w = h // 2, h % 2
                    po = DK * w
                    attn_ps = atp.tile([DK + 1, S], F32, tag="attn_ps")
                    for kb in range(NT):
                        qstart = kb * ID
                        for j in range(kb // 8, 2):
                            # scores tile covers q in [1024j, 1024j+1024)
                            c0 = 1024 * j
                            lo = max(qstart, c0)
                            sc = scp.tile([ID, 1024], F32, tag="sc")
                            qq = lo
                            while qq < c0 + 1024:
                                qe = min((qq // QB + 1) * QB, c0 + 1024)
                                nc.tensor.matmul(
                                    sc[:, qq - c0 : qe - c0],
                                    kT[p][po : po + DK,
                                          kb * ID : (kb + 1) * ID],
                                    qT[p][po : po + DK, qq:qe],
                                    start=True,
                                    stop=True,
                                )
                                qq = qe
                            ex = expp.tile([ID, 1024], F32R, tag="ex")
                            nc.scalar.activation(
                                ex[:, lo - c0 : 1024],
                                sc[:, lo - c0 : 1024],
                                AF.Exp,
                                scale=0.125,
                            )
                            if c0 <= qstart:
                                nc.vector.tensor_tensor(
                                    out=ex[:, lo - c0 : lo - c0 + ID],
                                    in0=ex[:, lo - c0 : lo - c0 + ID],
                                    in1=trimask[:],
                                    op=MUL,
                                )
                            qq = lo
                            while qq < c0 + 1024:
                                qe = min((qq // QB + 1) * QB, c0 + 1024)
                                nc.tensor.matmul(
                                    attn_ps[:, qq:qe],
                                    vaug[:, kb, 65 * h : 65 * h + DK + 1],
                                    ex[:, qq - c0 : qe - c0],
                                    start=(kb == 0),
                                    stop=(kb == NT - 1),
                                    skip_group_check=True,
                                )
                                qq = qe

                    # normalize rows 0..63 by row 64 (denominator)
                    den = denp.tile([1, S], F32R, tag="den")
                    nc.scalar.copy(den[:], attn_ps[DK : DK + 1, :])
                    dst_all = attnT_sb[p] if w == 0 else odd_tmp
                    for qb in range(NQB):
                        qs = slice(qb * QB, (qb + 1) * QB)
                        bc = scp.tile([DK, QB], F32, tag="sc")
                        nc.tensor.matmul(
                            bc[:], ones_r[:], den[:, qs], start=True, stop=True
                        )
                        rb = recbp.tile([DK, QB], F32, tag="rb")
                        nc.vector.reciprocal_approx_fast(rb[:], bc[:])
                        nc.vector.tensor_tensor(
                            out=dst_all[0:DK, qs],
                            in0=attn_ps[0:DK, qs],
                            in1=rb[:],
                            op=MUL,
                        )
                    if w == 1:
                        # shift odd head up to partitions 64-127
                        nc.gpsimd.dma_start(
                            attnT_sb[p][DK:ID, :], odd_tmp[0:DK, :]
                        )

            # ---------------- phase 4: output projection ------------------
            with tc.tile_pool(name="osb", bufs=3) as osbp, \
                 tc.tile_pool(name="ops", bufs=2, space="PSUM") as opsp:
                for mt in range(NT):
                    ts = slice(mt * ID, (mt + 1) * ID)
                    for nb in range(2):
                        ns = slice(nb * QB, (nb + 1) * QB)
                        o_ps = opsp.tile([ID, QB], F32, tag="o_ps")
                        for p in range(2):
                            nc.tensor.matmul(
                                o_ps[:],
                                attnT_sb[p][:, ts],
                                wo_sb[p][:, ns],
                                start=(p == 0),
                                stop=(p == 1),
                            )
                        ob = osbp.tile([ID, QB], F32, tag="ob")
                        if nb == 0:
                            nc.scalar.copy(ob[:], o_ps[:])
                        else:
                            nc.vector.tensor_copy(ob[:], o_ps[:])
                        nc.sync.dma_start(out[ts, ns], ob[:])

    nc.compile()
    return nc


def _get_prog():
    global _prog
    if _prog is None:
        _prog = _build_program()
    return _prog


def _host_prep(x, w_q, w_k, w_v, w_o, sin_tab, cos_tab, token_positions):
    """Build the 8 per-core input maps."""
    x = np.asarray(x, dtype=np.float32)
    w_q = np.asarray(w_q, dtype=np.float32)
    w_k = np.asarray(w_k, dtype=np.float32)
    w_v = np.asarray(w_v, dtype=np.float32)
    w_o = np.asarray(w_o, dtype=np.float32)
    sin_tab = np.asarray(sin_tab, dtype=np.float32)
    cos_tab = np.asarray(cos_tab, dtype=np.float32)
    tpos = np.asarray(token_positions).astype(np.int64)

    # per-head row permutation: even rope lanes first, then odd
    perm = np.concatenate([np.arange(0, DK, 2), np.arange(1, DK, 2)])

    xT_b = [np.ascontiguousarray(x[b].T) for b in range(B)]
    cs_b = []
    for b in range(B):
        cosg = cos_tab[tpos[b]]             # [S, 32]
        sing = sin_tab[tpos[b]]
        cs_b.append(
            (
                np.ascontiguousarray(np.tile(cosg, (1, HPC))),
                np.ascontiguousarray(np.tile(sing, (1, HPC))),
            )
        )

    trimask = np.triu(np.ones((ID, ID), dtype=np.float32))

    in_maps = []
    for c in range(NCORES):
        b, g = divmod(c, HPC)
        rows = np.arange(g * F, (g + 1) * F)
        rows_qk = (rows.reshape(HPC, DK)[:, perm]).reshape(-1)
        wq_t = w_q[rows_qk].T               # [D, F]
        wk_t = w_k[rows_qk].T
        wv_t = w_v[rows].T
        wqkv_c = np.ascontiguousarray(
            np.concatenate([wq_t, wk_t, wv_t], axis=1)
        )
        wo_c = np.ascontiguousarray(w_o[:, rows].T)  # [F, D]
        in_maps.append(
            {
                "xT": xT_b[b],
                "wqkv": wqkv_c,
                "wo01": np.ascontiguousarray(wo_c[0:ID]),
                "wo23": np.ascontiguousarray(wo_c[ID : 2 * ID]),
                "cosr": cs_b[b][0],
                "sinr": cs_b[b][1],
                "trimask": trimask,
            }
        )
    return in_maps


def kernel(x, w_q, w_k, w_v, w_o, sin_tab, cos_tab, token_positions):
    from concourse.bass_utils import run_bass_kernel_spmd

    nc = _get_prog()
    in_maps = _host_prep(
        x, w_q, w_k, w_v, w_o, sin_tab, cos_tab, token_positions
    )
    res = run_bass_kernel_spmd(nc, in_maps, list(range(NCORES)))
    outs = [np.asarray(res.results[c]["out"]) for c in range(NCORES)]
    full = np.empty((B, S, D), dtype=np.float32)
    for b in range(B):
        full[b] = outs[b * HPC]
        for g in range(1, HPC):
            full[b] += outs[b * HPC + g]
    return full


if __name__ == "__main__":
    import reference

    inputs = {k: np.asarray(v) for k, v in reference.setup_inputs().items()}
    got = kernel(**inputs)
    exp = np.asarray(reference.reference(**reference.setup_inputs()))
    err = np.abs(got - exp).max() / np.abs(exp).max()
    print("Relative error:", err)


# revision 9
# speedup vs baseline: 1.2508x; 1.0153x over previous
"""Trainium2 Bass kernel for multi-head self-attention with RoPE (causal).

Problem: B=2, S=2048, D=1024, H=16 heads, dk=64, fp32.

Sharding (8 NeuronCores): core c handles batch b = c//4 and head group
g = c%4 (heads 4g..4g+3).  Data parallel over batch, tensor parallel over
heads: w_q/w_k/w_v rows and w_o columns are split by head group; each core
produces a partial output [S, D] (its heads' contribution through w_o) and
the host sums the 4 partials per batch.

Device algorithm per core (all matmuls in float32r — fp32 bits, reduced
precision multiplier, 4x the fp32 matmul rate):
  1. q|k|v projections token-major: out[128 tok, 512/256] tiles, contracting
     over D in 8 chunks of 128.  q/k weight rows are pre-permuted per head
     (even pairs first) so RoPE uses contiguous halves.
  2. RoPE applied in token-major layout on DVE straight out of PSUM, then
     PE-transposed to feature-major qT/kT [64 feats, S] per head.
  3. Attention per head with scores TRANSPOSED: scoresT[k, q] tiles
     (k on partitions), exp on ScalarE (scale=1/8 folded in, no max pass —
     inputs are scale-bounded), causal via block skipping + one triangular
     mask multiply on diagonal blocks, then attnT[dk, q] accumulated in
     PSUM with contraction over k.  A ones-row appended to V gives the
     softmax denominator as row 64 of the same accumulation.
  4. Normalize: PE outer-product broadcast of the denominator row,
     reciprocal_approx_fast on the broadcast, multiplied into the
     PSUM->SBUF copy of attnT.  Odd heads are shifted to partitions 64-127
     by an SBUF->SBUF DMA so the output projection contracts K=128.
  5. Output projection from packed attnT pairs (feature-major is exactly
     the lhsT the PE wants): out[128 tok, 512] accumulating 2 head pairs.
"""

import sys

sys.path.insert(0, "/opt/trn_rl_repo")

import numpy as np

B, S, D, H, DK = 2, 2048, 1024, 16, 64
NCORES = 8
HPC = 4            # heads per core
F = HPC * DK       # 256 features per core
ID = 128           # partition tile
NT = S // ID       # 16 token tiles
KC = D // ID       # 8 contraction chunks
QB = 512           # PSUM bank width in fp32
NQB = S // QB      # 4
XW = 512           # x DMA tile width (tokens)

_prog = None


def _build_program():
    import concourse.bass as bass
    import concourse.tile as tile
    from concourse import bacc, mybir
    from concourse.masks import make_identity

    F32 = mybir.dt.float32
    F32R = mybir.dt.float32r
    AF = mybir.ActivationFunctionType
    MUL = mybir.AluOpType.mult

    nc = bacc.Bacc("TRN2", target_bir_lowering=False, debug=False)

    xT = nc.declare_dram_parameter("xT", [D, S], F32R, isOutput=False)
    wqkv = nc.declare_dram_parameter("wqkv", [D, 3 * F], F32R, isOutput=False)
    wo01 = nc.declare_dram_parameter("wo01", [ID, D], F32R, isOutput=False)
    wo23 = nc.declare_dram_parameter("wo23", [ID, D], F32R, isOutput=False)
    cosr = nc.declare_dram_parameter("cosr", [S, ID], F32, isOutput=False)
    sinr = nc.declare_dram_parameter("sinr", [S, ID], F32, isOutput=False)
    trim = nc.declare_dram_parameter("trimask", [ID, ID], F32R, isOutput=False)
    out = nc.declare_dram_parameter("out", [S, D], F32, isOutput=True)

    with nc.allow_low_precision(reason="f32r matmul pipeline by design"), \
         tile.TileContext(nc) as tc:
        # ---------------- persistent pools ----------------
        with tc.tile_pool(name="const", bufs=1) as constp, \
             tc.tile_pool(name="wo", bufs=1) as wop, \
             tc.tile_pool(name="qkT", bufs=1) as qkTp, \
             tc.tile_pool(name="vaug", bufs=1) as vaugp, \
             tc.tile_pool(name="attnT", bufs=1) as attnTp:

            ident = constp.tile([ID, ID], F32, tag="ident")
            make_identity(nc, ident[:])
            trimask = constp.tile([ID, ID], F32R, tag="trimask")
            nc.sync.dma_start(trimask[:], trim[:])
            ones_f = constp.tile([1, DK], F32, tag="ones_f")
            nc.vector.memset(ones_f[:], 1.0)
            ones_r = constp.tile([1, DK], F32R, tag="ones_r")
            nc.vector.tensor_copy(ones_r[:], ones_f[:])
            onecol_f = constp.tile([ID, NT], F32, tag="onecol")
            nc.vector.memset(onecol_f[:], 1.0)

            wo_sb = [
                wop.tile([ID, D], F32R, tag=f"wo{p}", name=f"wo_sb{p}")
                for p in range(2)
            ]
            nc.sync.dma_start(wo_sb[0][:], wo01[:])
            nc.sync.dma_start(wo_sb[1][:], wo23[:])

            # feature-major q/k, one tensor per head pair; head h lives on
            # partitions 64*(h%2) .. of tensor h//2
            qT = [
                qkTp.tile([ID, S], F32R, tag=f"qT{p}", name=f"qT{p}")
                for p in range(2)
            ]
            kT = [
                qkTp.tile([ID, S], F32R, tag=f"kT{p}", name=f"kT{p}")
                for p in range(2)
            ]

            # token-major v with a ones column per head: head h in columns
            # [65h, 65h+64], ones at column 65h+64
            vaug = vaugp.tile([ID, NT, HPC * (DK + 1)], F32R, tag="vaug")
            for h in range(HPC):
                nc.vector.tensor_copy(vaug[:, :, 65 * h + DK], onecol_f[:])

            # packed attnT: pair p holds head 2p on partitions 0-63 and
            # head 2p+1 on partitions 64-127
            attnT_sb = [
                attnTp.tile([ID, S], F32R, tag=f"attnT{p}", name=f"attnT{p}")
                for p in range(2)
            ]
            # odd heads normalize into a low-partition temp, then an
            # SBUF->SBUF DMA shifts them up to partitions 64-127
            odd_tmp = attnTp.tile([DK, S], F32R, tag="odd_tmp")

            # ---------------- phase 1+2: projections, rope, transposes ----
            with tc.tile_pool(name="wqkv", bufs=1) as wqkvp, \
                 tc.tile_pool(name="cs", bufs=1) as csp, \
                 tc.tile_pool(name="xtile", bufs=16) as xtp, \
                 tc.tile_pool(name="rot", bufs=3) as rotp, \
                 tc.tile_pool(name="tmp", bufs=4) as tmpp, \
                 tc.tile_pool(name="proj_ps", bufs=2, space="PSUM") as pjp, \
                 tc.tile_pool(name="vps", bufs=2, space="PSUM") as vpp, \
                 tc.tile_pool(name="tp_ps", bufs=3, space="PSUM") as tpp:

                wqkv_sb = wqkvp.tile([ID, KC, 3 * F], F32R, tag="wqkv")
                nc.sync.dma_start(
                    wqkv_sb[:],
                    wqkv[:].rearrange("(c p) f -> p c f", p=ID),
                )
                cos_sb = csp.tile([ID, NT, HPC, 32], F32, tag="cos")
                sin_sb = csp.tile([ID, NT, HPC, 32], F32, tag="sin")
                nc.sync.dma_start(
                    cos_sb[:].rearrange("p t h d -> p t (h d)"),
                    cosr[:].rearrange("(t p) f -> p t f", p=ID),
                )
                nc.sync.dma_start(
                    sin_sb[:].rearrange("p t h d -> p t (h d)"),
                    sinr[:].rearrange("(t p) f -> p t f", p=ID),
                )

                xts = {}
                for mt in range(NT):
                    ts = slice(mt * ID, (mt + 1) * ID)
                    qk_ps = pjp.tile([ID, 2, HPC, DK], F32, tag="qk_ps")
                    v_ps = vpp.tile([ID, F], F32, tag="v_ps")
                    sub = mt % (XW // ID)
                    for kc in range(KC):
                        if sub == 0:
                            xt = xtp.tile([ID, XW], F32R, tag="xt",
                                          name=f"xt{mt}_{kc}")
                            nc.gpsimd.dma_start(
                                xt[:],
                                xT[kc * ID : (kc + 1) * ID,
                                   mt * ID : mt * ID + XW],
                            )
                            xts[kc] = xt
                        xsl = xts[kc][:, sub * ID : (sub + 1) * ID]
                        nc.tensor.matmul(
                            qk_ps[:].rearrange("p a h d -> p (a h d)"),
                            xsl,
                            wqkv_sb[:, kc, 0 : 2 * F],
                            start=(kc == 0),
                            stop=(kc == KC - 1),
                        )
                        nc.tensor.matmul(
                            v_ps[:],
                            xsl,
                            wqkv_sb[:, kc, 2 * F : 3 * F],
                            start=(kc == 0),
                            stop=(kc == KC - 1),
                        )

                    ct = cos_sb[:, mt]
                    st = sin_sb[:, mt]
                    # RoPE on DVE: halves are contiguous (weights were
                    # pre-permuted even-pairs-first per head)
                    for a in range(2):          # 0 = q, 1 = k
                        src = qk_ps[:, a]       # [ID, HPC, DK] psum
                        rot = rotp.tile([ID, HPC, DK], F32, tag=f"rot{a}")
                        t1 = tmpp.tile([ID, HPC, 32], F32, tag=f"t1{a}")
                        t2 = tmpp.tile([ID, HPC, 32], F32, tag=f"t2{a}")
                        top = src[:, :, 0:32]
                        bot = src[:, :, 32:64]
                        nc.vector.tensor_tensor(
                            out=rot[:, :, 0:32], in0=top, in1=ct, op=MUL
                        )
                        nc.vector.tensor_tensor(
                            out=t1[:], in0=bot, in1=st, op=MUL
                        )
                        nc.vector.tensor_sub(
                            rot[:, :, 0:32], rot[:, :, 0:32], t1[:]
                        )
                        nc.vector.tensor_tensor(
                            out=rot[:, :, 32:64], in0=top, in1=st, op=MUL
                        )
                        nc.vector.tensor_tensor(
                            out=t2[:], in0=bot, in1=ct, op=MUL
                        )
                        nc.vector.tensor_add(
                            rot[:, :, 32:64], rot[:, :, 32:64], t2[:]
                        )
                        # transpose to feature-major, per head pair
                        for p in range(2):
                            tp = tpp.tile([ID, ID], F32, tag="tp")
                            nc.tensor.transpose(
                                tp[:],
                                rot[:, 2 * p : 2 * p + 2, :],
                                ident[:],
                            )
                            dst = (qT if a == 0 else kT)[p][:, ts]
                            if a == 0:
                                nc.scalar.copy(dst, tp[:])
                            else:
                                nc.vector.tensor_copy(dst, tp[:])

                    # v copies into augmented layout (ACT)
                    for h in range(HPC):
                        nc.scalar.copy(
                            vaug[:, mt, 65 * h : 65 * h + DK],
                            v_ps[:, DK * h : DK * h + DK],
                        )

            # ---------------- phase 3: attention per head ----------------
            # Each head runs as TWO independent chains (q-halves j=0/j=1)
            # with separate 2-bank PSUM accumulators, so the PE always has
            # an independent chunk to stream while ScalarE runs exp on the
            # other chain.
            with tc.tile_pool(name="expT", bufs=3) as expp, \
                 tc.tile_pool(name="den", bufs=2) as denp, \
                 tc.tile_pool(name="recb", bufs=2) as recbp, \
                 tc.tile_pool(name="sc_ps", bufs=2, space="PSUM") as scp, \
                 tc.tile_pool(name="attn_ps", bufs=1, space="PSUM") as atp:

                for h in range(HPC):
                    p, w = h // 2, h % 2
                    po = DK * w

                    def chunk(kb, j, aps, last):
                        """scores -> exp -> (mask) -> attn for q-block j."""
                        qstart = kb * ID
                        c0 = 1024 * j
                        lo = max(qstart, c0)
                        sc = scp.tile([ID, 1024], F32, tag="sc",
                                      name=f"sc{h}_{kb}_{j}")
                        qq = lo
                        while qq < c0 + 1024:
                            qe = min((qq // QB + 1) * QB, c0 + 1024)
                            nc.tensor.matmul(
                                sc[:, qq - c0 : qe - c0],
                                kT[p][po : po + DK,
                                      kb * ID : (kb + 1) * ID],
                                qT[p][po : po + DK, qq:qe],
                                start=True,
                                stop=True,
                            )
                            qq = qe
                        ex = expp.tile([ID, 1024], F32R, tag="ex",
                                       name=f"ex{h}_{kb}_{j}")
                        nc.scalar.activation(
                            ex[:, lo - c0 : 1024],
                            sc[:, lo - c0 : 1024],
                            AF.Exp,
                            scale=0.125,
                        )
                        if c0 <= qstart:
                            nc.vector.tensor_tensor(
                                out=ex[:, lo - c0 : lo - c0 + ID],
                                in0=ex[:, lo - c0 : lo - c0 + ID],
                                in1=trimask[:],
                                op=MUL,
                            )
                        qq = lo
                        while qq < c0 + 1024:
                            qe = min((qq // QB + 1) * QB, c0 + 1024)
                            nc.tensor.matmul(
                                aps[:, qq - c0 : qe - c0],
                                vaug[:, kb, 65 * h : 65 * h + DK + 1],
                                ex[:, qq - c0 : qe - c0],
                                start=(kb == 0),
                                stop=last,
                                skip_group_check=True,
                            )
                            qq = qe

                    apsA = atp.tile([DK + 1, 1024], F32, tag="attnA",
                                    name=f"attnA{h}")
                    apsB = atp.tile([DK + 1, 1024], F32, tag="attnB",
                                    name=f"attnB{h}")
                    # interleave the two chains so PE never waits on exp
                    for kb in range(NT):
                        chunk(kb, 1, apsB, kb == NT - 1)
                        if kb < 8:
                            chunk(kb, 0, apsA, kb == 7)

                    # normalize rows 0..63 by row 64 (denominator)
                    dst_all = attnT_sb[p] if w == 0 else odd_tmp
                    for j, aps in ((0, apsA), (1, apsB)):
                        den = denp.tile([1, 1024], F32R, tag="den",
                                        name=f"den{h}_{j}")
                        nc.scalar.copy(den[:], aps[DK : DK + 1, :])
                        for qb in range(2):
                            qs = slice(qb * QB, (qb + 1) * QB)
                            gqs = slice(1024 * j + qb * QB,
                                        1024 * j + (qb + 1) * QB)
                            bc = scp.tile([DK, QB], F32, tag="sc",
                                          name=f"bc{h}_{j}_{qb}")
                            nc.tensor.matmul(
                                bc[:], ones_r[:], den[:, qs],
                                start=True, stop=True,
                            )
                            rb = recbp.tile([DK, QB], F32, tag="rb",
                                            name=f"rb{h}_{j}_{qb}")
                            nc.vector.reciprocal_approx_fast(rb[:], bc[:])
                            nc.vector.tensor_tensor(
                                out=dst_all[0:DK, gqs],
                                in0=aps[0:DK, qs],
                                in1=rb[:],
                                op=MUL,
                            )
                    if w == 1:
                        # shift odd head up to partitions 64-127
                        nc.gpsimd.dma_start(
                            attnT_sb[p][DK:ID, :], odd_tmp[0:DK, :]
                        )

            # ---------------- phase 4: output projection ------------------
            with tc.tile_pool(name="osb", bufs=3) as osbp, \
                 tc.tile_pool(name="ops", bufs=5, space="PSUM") as opsp:
                for mt in range(NT):
                    ts = slice(mt * ID, (mt + 1) * ID)
                    for nb in range(2):
                        ns = slice(nb * QB, (nb + 1) * QB)
                        o_ps = opsp.tile([ID, QB], F32, tag="o_ps")
                        for p in range(2):
                            nc.tensor.matmul(
                                o_ps[:],
                                attnT_sb[p][:, ts],
                                wo_sb[p][:, ns],
                                start=(p == 0),
                                stop=(p == 1),
                            )
                        ob = osbp.tile([ID, QB], F32, tag="ob")
                        if nb == 0:
                            nc.scalar.copy(ob[:], o_ps[:])
                        else:
                            nc.vector.tensor_copy(ob[:], o_ps[:])
                        nc.sync.dma_start(out[ts, ns], ob[:])

    nc.compile()
    return nc


def _get_prog():
    global _prog
    if _prog is None:
        _prog = _build_program()
    return _prog


def _host_prep(x, w_q, w_k, w_v, w_o, sin_tab, cos_tab, token_positions):
    """Build the 8 per-core input maps."""
    x = np.asarray(x, dtype=np.float32)
    w_q = np.asarray(w_q, dtype=np.float32)
    w_k = np.asarray(w_k, dtype=np.float32)
    w_v = np.asarray(w_v, dtype=np.float32)
    w_o = np.asarray(w_o, dtype=np.float32)
    sin_tab = np.asarray(sin_tab, dtype=np.float32)
    cos_tab = np.asarray(cos_tab, dtype=np.float32)
    tpos = np.asarray(token_positions).astype(np.int64)

    # per-head row permutation: even rope lanes first, then odd
    perm = np.concatenate([np.arange(0, DK, 2), np.arange(1, DK, 2)])

    xT_b = [np.ascontiguousarray(x[b].T) for b in range(B)]
    cs_b = []
    for b in range(B):
        cosg = cos_tab[tpos[b]]             # [S, 32]
        sing = sin_tab[tpos[b]]
        cs_b.append(
            (
                np.ascontiguousarray(np.tile(cosg, (1, HPC))),
                np.ascontiguousarray(np.tile(sing, (1, HPC))),
            )
        )

    trimask = np.triu(np.ones((ID, ID), dtype=np.float32))

    in_maps = []
    for c in range(NCORES):
        b, g = divmod(c, HPC)
        rows = np.arange(g * F, (g + 1) * F)
        rows_qk = (rows.reshape(HPC, DK)[:, perm]).reshape(-1)
        wq_t = w_q[rows_qk].T               # [D, F]
        wk_t = w_k[rows_qk].T
        wv_t = w_v[rows].T
        wqkv_c = np.ascontiguousarray(
            np.concatenate([wq_t, wk_t, wv_t], axis=1)
        )
        wo_c = np.ascontiguousarray(w_o[:, rows].T)  # [F, D]
        in_maps.append(
            {
                "xT": xT_b[b],
                "wqkv": wqkv_c,
                "wo01": np.ascontiguousarray(wo_c[0:ID]),
                "wo23": np.ascontiguousarray(wo_c[ID : 2 * ID]),
                "cosr": cs_b[b][0],
                "sinr": cs_b[b][1],
                "trimask": trimask,
            }
        )
    return in_maps


def kernel(x, w_q, w_k, w_v, w_o, sin_tab, cos_tab, token_positions):
    from concourse.bass_utils import run_bass_kernel_spmd

    nc = _get_prog()
    in_maps = _host_prep(
        x, w_q, w_k, w_v, w_o, sin_tab, cos_tab, token_positions
    )
    res = run_bass_kernel_spmd(nc, in_maps, list(range(NCORES)))
    outs = [np.asarray(res.results[c]["out"]) for c in range(NCORES)]
    full = np.empty((B, S, D), dtype=np.float32)
    for b in range(B):
        full[b] = outs[b * HPC]
        for g in range(1, HPC):
            full[b] += outs[b * HPC + g]
    return full


if __name__ == "__main__":
    import reference

    inputs = {k: np.asarray(v) for k, v in reference.setup_inputs().items()}
    got = kernel(**inputs)
    exp = np.asarray(reference.reference(**reference.setup_inputs()))
    err = np.abs(got - exp).max() / np.abs(exp).max()
    print("Relative error:", err)
